# revision 2
# baseline (speedup 1.0000x reference)
"""Trainium2 Bass kernel for nn_CenterLossNet (center-loss softmax over classes).

Math (reference):
    f = l2_normalize(features); c = l2_normalize(centers)
    dis[n,k]  = -5 * (|f_n|^2 + |c_k|^2 - 2 f_n.c_k)        # [N, C]
    pos[n]    = dis[n, labels[n]] + bias[labels[n]]
    den[n]    = sum_k exp(dis[n,k]) - exp(dis[n,l_n]) + exp(pos[n])
    loss      = mean(log(den) - pos) + var(pos, ddof=1);  returns (loss, var)

Device does the heavy part: S = f_hat @ c_hat.T (8192x10000x512 matmul in
fp8e4m3 DoubleRow perf mode, operands pre-scaled by 2^9) fused with the
row-sum of exp(10*S + ab_n).  The exp+sum of each [128, 2048] PSUM megatile
goes to one of two engines so neither is the bottleneck (the PE matmul is):

  - ~2/3 of megatiles: scalar-engine ACTIVATE Exp with accum_out (the
    row-sum accumulates for free during the activation; +1 accumulator
    read per tile).
  - ~1/3 of megatiles: vector-engine Schraudolph fast-exp - one
    tensor_scalar writes int32(A*dis + B) (whose bit pattern read as fp32
    approximates exp(dis) to ~2%), a second tensor_scalar row-sums the
    bitcast tile.  The systematic (1+f)/2^f excess is exactly
    E = 1/(2 ln^2 2) under the (empirically uniform) mantissa-fraction
    distribution, so the host divides those partial sums by R; the residual
    per-row fluctuation is ~1e-4 relative.

Everything O(N) or O(C) runs on host in fp64, so pos/variance use exact
fp32 inputs.  Sharding: data-parallel over batch N across 8 cores; centers
replicated.  The per-class |c_k|^2 term is folded as exactly 1.0 with a
host-side mean-residual correction; pos[n] uses the exact per-label norms.
"""

import numpy as np
import ml_dtypes

import concourse.bacc as bacc
import concourse.mybir as mybir
import concourse.tile as tile
from concourse.bass_utils import run_bass_kernel_spmd

N, C, D = 8192, 10000, 512
N_CORES = 8
NS = N // N_CORES       # 1024 rows per core
P = 128                 # partitions
M_TILES = NS // P       # 8 row tiles per core
K2 = D // (2 * P)       # 2 DoubleRow contraction tiles (256 rows each)
CW = 512                # matmul free-dim tile (one PSUM bank of fp32)
GW = 2048               # PSUM megatile width: 4 banks, one consumer op each
G_TILES = (C + GW - 1) // GW  # 5 (4 x 2048 + 1808)
N_TILES = G_TILES * M_TILES   # 40 megatiles per core
SCALE = 5.0
EPS = 1e-12
FP8_SCALE = 512.0       # 2^9: keeps |values| <= ~120 within e4m3 normal range
FP8 = ml_dtypes.float8_e4m3
DVE_MOD = 3             # megatile idx % 3 == 2 -> vector-engine fast-exp path

# Schraudolph fast-exp constants: int32(A*x + B) bitcast to fp32 ~= exp(x).
A_EXP = float(2.0**23 / np.log(2.0))
B_EXP = float(127 * 2**23)
R_EXP = float(1.0 / (2.0 * np.log(2.0) ** 2))   # E[(1+f)/2^f], f ~ U[0,1)

_compiled = None
LAST_RESULTS = None


def _is_dve(idx: int) -> bool:
    return idx % DVE_MOD == DVE_MOD - 1


def _build():
    nc = bacc.Bacc(
        "TRN2",
        target_bir_lowering=False,
        debug=False,
        enable_asserts=False,
        num_devices=N_CORES,
    )
    # strip 0 is stored as four 512-column chunks (2 KB per partition each)
    # so the first matmuls can start as soon as chunk 0 lands
    c0_d = [
        nc.dram_tensor(f"c0{j}", [P, K2, 2, CW], mybir.dt.float8e4, kind="ExternalInput").ap()
        for j in range(GW // CW)
    ]
    ct_d = nc.dram_tensor(
        "ct", [G_TILES - 1, P, K2, 2, GW], mybir.dt.float8e4, kind="ExternalInput"
    ).ap()
    ft_d = nc.dram_tensor(
        "ft", [P, K2, 2, NS], mybir.dt.float8e4, kind="ExternalInput"
    ).ap()
    # ab[:, 0, m] = ACT exp bias; ab[:, 1, m] = A*ab + B (Schraudolph bias)
    ab_d = nc.dram_tensor("ab", [P, 2, M_TILES], mybir.dt.float32, kind="ExternalInput").ap()
    # per-(g,m) partial row-sums (col = g*M + m); host does the final combine
    rs_d = nc.dram_tensor(
        "rs", [P, N_TILES], mybir.dt.float32, kind="ExternalOutput"
    ).ap()

    with tile.TileContext(nc) as tc:
        with (
            tc.tile_pool(name="cpool", bufs=1) as cpool,
            tc.tile_pool(name="spool", bufs=1) as spool,
            tc.tile_pool(name="epool", bufs=2) as epool,
            tc.tile_pool(name="ipool", bufs=2) as ipool,
            tc.tile_pool(name="ppool", bufs=2, space="PSUM") as ppool,
        ):
            # warm the PE clock (HAM) with throwaway DoubleRow matmuls on a
            # zeroed tile while the first input DMAs are in flight
            z8 = spool.tile([P, 2, CW], mybir.dt.float8e4, tag="z8")
            nc.gpsimd.memset(z8[:], 0.0)
            wps = ppool.tile([P, CW], mybir.dt.float32, tag="ps", name="wps")
            for _ in range(12):
                nc.tensor.matmul(
                    wps[:],
                    z8[:, :, 0:P],
                    z8[:],
                    start=True,
                    stop=True,
                    perf_mode=mybir.MatmulPerfMode.DoubleRow,
                    skip_group_check=True,
                )

            # critical prefix on the sync ring, in first-use order
            ft_sb = cpool.tile([P, K2, 2, NS], mybir.dt.float8e4, tag="ft")
            nc.sync.dma_start(out=ft_sb[:], in_=ft_d)

            c0_sb = []
            for j in range(GW // CW):
                t = cpool.tile([P, K2, 2, CW], mybir.dt.float8e4, tag=f"c0{j}")
                c0_sb.append(t)
            nc.sync.dma_start(out=c0_sb[0][:], in_=c0_d[0])
            nc.sync.dma_start(out=c0_sb[1][:], in_=c0_d[1])

            ab_sb = spool.tile([P, 2, M_TILES], mybir.dt.float32, tag="ab")
            nc.sync.dma_start(out=ab_sb[:], in_=ab_d)

            nc.sync.dma_start(out=c0_sb[2][:], in_=c0_d[2])
            nc.sync.dma_start(out=c0_sb[3][:], in_=c0_d[3])

            # remaining strips: one DMA per strip, FIFO behind the prefix
            ct_sb = [None]
            for g in range(1, G_TILES):
                gw = min(GW, C - g * GW)
                t = cpool.tile(
                    [P, K2, 2, GW], mybir.dt.float8e4, tag=f"ct{g}", name=f"ct{g}"
                )
                nc.sync.dma_start(out=t[:, :, :, :gw], in_=ct_d[g - 1][:, :, :, :gw])
                ct_sb.append(t)

            parts = spool.tile([P, N_TILES], mybir.dt.float32, tag="parts")

            act_scale = 2.0 * SCALE / (FP8_SCALE * FP8_SCALE)
            dve_a = A_EXP * act_scale

            for g in range(G_TILES):
                gw = min(GW, C - g * GW)
                n_sl = (gw + CW - 1) // CW
                for m in range(M_TILES):
                    tile_idx = g * M_TILES + m
                    ps = ppool.tile([P, GW], mybir.dt.float32, tag="ps")
                    for k in range(K2):
                        for j in range(n_sl):
                            w = min(CW, gw - j * CW)
                            if g == 0:
                                rhs = c0_sb[j][:, k, :, :w]
                            else:
                                rhs = ct_sb[g][:, k, :, j * CW : j * CW + w]
                            nc.tensor.matmul(
                                ps[:, j * CW : j * CW + w],
                                ft_sb[:, k, :, m * P : (m + 1) * P],
                                rhs,
                                start=(k == 0),
                                stop=(k == K2 - 1),
                                perf_mode=mybir.MatmulPerfMode.DoubleRow,
                                skip_group_check=True,
                            )
                    acc_ap = parts[:, tile_idx : tile_idx + 1]
                    if _is_dve(tile_idx):
                        it = ipool.tile([P, GW], mybir.dt.int32, tag="it")
                        nc.vector.tensor_scalar(
                            it[:, :gw],
                            ps[:, :gw],
                            dve_a,
                            ab_sb[:, 1, m : m + 1],
                            op0=mybir.AluOpType.mult,
                            op1=mybir.AluOpType.add,
                        )
                        itf = it[:, :gw].bitcast(mybir.dt.float32)
                        nc.vector.tensor_scalar(
                            itf,
                            itf,
                            1.0,
                            None,
                            op0=mybir.AluOpType.mult,
                            op1=mybir.AluOpType.add,
                            accum_out=acc_ap,
                        )
                    else:
                        et = epool.tile([P, GW], mybir.dt.bfloat16, tag="exp")
                        nc.scalar.activation(
                            et[:, :gw],
                            ps[:, :gw],
                            mybir.ActivationFunctionType.Exp,
                            bias=ab_sb[:, 0, m : m + 1],
                            scale=act_scale,
                            accum_out=acc_ap,
                        )
            nc.sync.dma_start(out=rs_d, in_=parts[:])

    nc.compile()
    return nc


def _get_compiled():
    global _compiled
    if _compiled is None:
        _compiled = _build()
    return _compiled


def _l2n(x):
    n = np.sqrt(np.einsum("nd,nd->n", x, x, dtype=np.float32), dtype=np.float32)
    xh = x / np.maximum(n, np.float32(EPS))[:, None]
    sq = np.einsum("nd,nd->n", xh, xh, dtype=np.float32)
    return xh.astype(np.float32), sq.astype(np.float32)


def _pack_dr(xt):
    """[D, W] fp32 (pre-scaled) -> DoubleRow fp8 [P, K2, 2, W]:
    row d = k*256 + i*128 + p  ->  out[p, k, i]."""
    d, w = xt.shape
    return np.ascontiguousarray(
        xt.reshape(K2, 2, P, w).transpose(2, 0, 1, 3)
    ).astype(FP8)


def _pack_ct(xt):
    """[D, C] fp32 (pre-scaled) -> (strip-0 chunks 4 x [P, K2, 2, CW],
    strips 1.. [G-1, P, K2, 2, GW], last zero-padded)."""
    c0 = [
        np.ascontiguousarray(_pack_dr(xt[:, j * CW : (j + 1) * CW]))
        for j in range(GW // CW)
    ]
    ctr = np.zeros((G_TILES - 1, P, K2, 2, GW), dtype=FP8)
    for g in range(1, G_TILES):
        gw = min(GW, C - g * GW)
        ctr[g - 1, :, :, :, :gw] = _pack_dr(xt[:, g * GW : g * GW + gw])
    return c0, ctr


def _combine_rs(rs):
    """[P, G*M] per-core output (col = tile_idx = g*M + m) -> per-row sums
    [NS] (n = m*128 + p).  Schraudolph columns are pre-divided by R_EXP."""
    scaled = rs.astype(np.float64)
    for idx in range(N_TILES):
        if _is_dve(idx):
            scaled[:, idx] /= R_EXP
    out = scaled.reshape(P, G_TILES, M_TILES).sum(axis=1)
    return out.T.reshape(NS)


def kernel(features, labels, centers, bias):
    features = np.asarray(features, dtype=np.float32)
    centers = np.asarray(centers, dtype=np.float32)
    bias = np.asarray(bias, dtype=np.float32)
    labels_i = np.asarray(labels).astype(np.int64)

    fh, f2 = _l2n(features)          # [N, D], [N]
    ch, c2 = _l2n(centers)           # [C, D], [C]

    c0_8, ct8 = _pack_ct(ch.T * np.float32(FP8_SCALE))
    abias_full = (-SCALE * (f2 + np.float32(1.0))).astype(np.float32)

    in_maps = []
    for i in range(N_CORES):
        sl = slice(i * NS, (i + 1) * NS)
        ft8 = _pack_dr(fh[sl].T * np.float32(FP8_SCALE))    # [P, K2, 2, NS]
        ab = np.ascontiguousarray(
            abias_full[sl].reshape(M_TILES, P).T
        )  # [P, M_TILES], n = m*128 + p
        ab2 = np.stack(
            [ab, (A_EXP * ab.astype(np.float64) + B_EXP).astype(np.float32)],
            axis=1,
        )  # [P, 2, M_TILES]
        im = {"ct": ct8, "ft": ft8, "ab": np.ascontiguousarray(ab2)}
        for j in range(GW // CW):
            im[f"c0{j}"] = c0_8[j]
        in_maps.append(im)

    nc = _get_compiled()
    global LAST_RESULTS
    LAST_RESULTS = run_bass_kernel_spmd(nc, in_maps, core_ids=list(range(N_CORES)))

    rowsum = np.concatenate(
        [_combine_rs(LAST_RESULTS.results[i]["rs"]) for i in range(N_CORES)]
    ).astype(np.float64)

    # residual correction for the |c_k|^2 ~= 1 fold (mean of exp(-5*(c2-1)))
    wmean = np.exp(-SCALE * (c2.astype(np.float64) - 1.0)).mean()
    rowsum *= wmean

    # exact per-row label terms (fp32 inputs, fp64 math)
    cl = ch[labels_i]                                        # [N, D]
    dot = np.einsum("nd,nd->n", fh.astype(np.float64), cl.astype(np.float64))
    dis_l = -SCALE * (f2.astype(np.float64) + c2[labels_i].astype(np.float64) - 2.0 * dot)
    pos = dis_l + bias[labels_i, 0].astype(np.float64)

    num = np.exp(pos)
    den = rowsum - np.exp(dis_l) + num
    logits = np.log(den) - pos
    variance = np.var(pos, ddof=1)
    loss = logits.mean() + variance
    return (np.float32(loss), np.float32(variance))


# revision 11
# speedup vs baseline: 1.1242x; 1.1242x over previous
"""Trainium2 Bass kernel for nn_CenterLossNet (center-loss softmax over classes).

Math (reference):
    f = l2_normalize(features); c = l2_normalize(centers)
    dis[n,k]  = -5 * (|f_n|^2 + |c_k|^2 - 2 f_n.c_k)        # [N, C]
    pos[n]    = dis[n, labels[n]] + bias[labels[n]]
    den[n]    = sum_k exp(dis[n,k]) - exp(dis[n,l_n]) + exp(pos[n])
    loss      = mean(log(den) - pos) + var(pos, ddof=1);  returns (loss, var)

Device does the heavy part: S = f_hat @ c_hat.T (8192x10000x512 matmul in
fp8e4m3 DoubleRow perf mode, operands pre-scaled by 2^9) fused with the
row-sum of exp(10*S + ab_n).  The exp+sum of each [128, 2048] PSUM megatile
goes to one of two engines so neither is the bottleneck (the PE matmul is):

  - ~2/3 of megatiles: scalar-engine ACTIVATE Exp with accum_out (the
    row-sum accumulates for free during the activation; +1 accumulator
    read per tile).
  - ~1/3 of megatiles: vector-engine Schraudolph fast-exp - one
    tensor_scalar writes int32(A*dis + B) (whose bit pattern read as fp32
    approximates exp(dis) to ~2%), a second tensor_scalar row-sums the
    bitcast tile.  The systematic (1+f)/2^f excess is exactly
    E = 1/(2 ln^2 2) under the (empirically uniform) mantissa-fraction
    distribution, so the host divides those partial sums by R; the residual
    per-row fluctuation is ~1e-4 relative.

Everything O(N) or O(C) runs on host in fp64, so pos/variance use exact
fp32 inputs.  Sharding: data-parallel over batch N across 8 cores; centers
replicated.  The per-class |c_k|^2 term is folded as exactly 1.0 with a
host-side mean-residual correction; pos[n] uses the exact per-label norms.
"""

import numpy as np
import ml_dtypes

import concourse.bacc as bacc
import concourse.mybir as mybir
import concourse.tile as tile
from concourse.bass_utils import run_bass_kernel_spmd

N, C, D = 8192, 10000, 512
N_CORES = 8
NS = N // N_CORES       # 1024 rows per core
P = 128                 # partitions
M_TILES = NS // P       # 8 row tiles per core
K2 = D // (2 * P)       # 2 DoubleRow contraction tiles (256 rows each)
CW = 512                # matmul free-dim tile (one PSUM bank of fp32)
GW = 2048               # DRAM strip width (4 x 512-col chunks)
G_TILES = (C + GW - 1) // GW  # 5 (4 x 2048 + 1808)
BW = 1024               # PSUM block width: 2 banks; 4 blocks ping-pong in PSUM
B_COLS = (C + BW - 1) // BW   # 10 column blocks (9 x 1024 + 784)
N_BLOCKS = B_COLS * M_TILES   # 80 blocks per core
SCALE = 5.0
EPS = 1e-12
FP8_SCALE = 512.0       # 2^9: keeps |values| <= ~120 within e4m3 normal range
FP8 = ml_dtypes.float8_e4m3

# Schraudolph fast-exp constants: int32(A*x + B) bitcast to fp32 ~= exp(x).
A_EXP = float(2.0**23 / np.log(2.0))
B_EXP = float(127 * 2**23)
R_EXP = float(1.0 / (2.0 * np.log(2.0) ** 2))   # E[(1+f)/2^f], f ~ U[0,1)

_compiled = None
LAST_RESULTS = None

# Per-block consumer assignment: 34 of 80 blocks go to the DVE Schraudolph
# path (evenly spaced), the rest to the scalar-engine ACTIVATE path.  The
# DVE reduce streams the two halves of the bitcast tile through one
# scalar_tensor_tensor add whose accum_out sums both (half the elements
# streamed), keeping DVE within the PE-paced window.
N_DVE = 34
DVE_SET = frozenset(round(i * N_BLOCKS / N_DVE) for i in range(N_DVE))
assert len(DVE_SET) == N_DVE


def _is_dve(idx: int) -> bool:
    return idx in DVE_SET


def _build():
    nc = bacc.Bacc(
        "TRN2",
        target_bir_lowering=False,
        debug=False,
        enable_asserts=False,
        num_devices=N_CORES,
    )
    # strip 0 is stored as four 512-column chunks (2 KB per partition each)
    # so the first matmuls can start as soon as chunk 0 lands
    c0_d = [
        nc.dram_tensor(f"c0{j}", [P, K2, 2, CW], mybir.dt.float8e4, kind="ExternalInput").ap()
        for j in range(GW // CW)
    ]
    ct_d = nc.dram_tensor(
        "ct", [G_TILES - 1, P, K2, 2, GW], mybir.dt.float8e4, kind="ExternalInput"
    ).ap()
    ft_d = nc.dram_tensor(
        "ft", [P, K2, 2, NS], mybir.dt.float8e4, kind="ExternalInput"
    ).ap()
    # ab[:, 0, m] = ACT exp bias; ab[:, 1, m] = A*ab + B (Schraudolph bias)
    ab_d = nc.dram_tensor("ab", [P, 2, M_TILES], mybir.dt.float32, kind="ExternalInput").ap()
    # per-(b,m) partial row-sums (col = b*M + m); host does the final combine
    rs_d = nc.dram_tensor(
        "rs", [P, N_BLOCKS], mybir.dt.float32, kind="ExternalOutput"
    ).ap()

    with tile.TileContext(nc) as tc:
        with (
            tc.tile_pool(name="cpool", bufs=1) as cpool,
            tc.tile_pool(name="spool", bufs=1) as spool,
            tc.tile_pool(name="epool", bufs=1) as epool,
            tc.tile_pool(name="ipool", bufs=1) as ipool,
            tc.tile_pool(name="ppool", bufs=4, space="PSUM") as ppool,
        ):
            # warm the PE clock (HAM) with throwaway DoubleRow matmuls on a
            # zeroed tile while the first input DMAs are in flight
            z8 = spool.tile([P, 2, CW], mybir.dt.float8e4, tag="z8")
            nc.gpsimd.memset(z8[:], 0.0)
            wps = ppool.tile([P, CW], mybir.dt.float32, tag="ps", name="wps")
            for _ in range(12):
                nc.tensor.matmul(
                    wps[:],
                    z8[:, :, 0:P],
                    z8[:],
                    start=True,
                    stop=True,
                    perf_mode=mybir.MatmulPerfMode.DoubleRow,
                    skip_group_check=True,
                )

            # critical prefix on the sync ring, in first-use order
            ft_sb = cpool.tile([P, K2, 2, NS], mybir.dt.float8e4, tag="ft")
            nc.sync.dma_start(out=ft_sb[:], in_=ft_d)

            c0_sb = []
            for j in range(GW // CW):
                t = cpool.tile([P, K2, 2, CW], mybir.dt.float8e4, tag=f"c0{j}")
                c0_sb.append(t)
            nc.sync.dma_start(out=c0_sb[0][:], in_=c0_d[0])
            nc.sync.dma_start(out=c0_sb[1][:], in_=c0_d[1])

            ab_sb = spool.tile([P, 2, M_TILES], mybir.dt.float32, tag="ab")
            nc.sync.dma_start(out=ab_sb[:], in_=ab_d)

            nc.sync.dma_start(out=c0_sb[2][:], in_=c0_d[2])
            nc.sync.dma_start(out=c0_sb[3][:], in_=c0_d[3])

            # remaining strips: one DMA per strip, FIFO behind the prefix
            ct_sb = [None]
            for g in range(1, G_TILES):
                gw = min(GW, C - g * GW)
                t = cpool.tile(
                    [P, K2, 2, GW], mybir.dt.float8e4, tag=f"ct{g}", name=f"ct{g}"
                )
                nc.sync.dma_start(out=t[:, :, :, :gw], in_=ct_d[g - 1][:, :, :, :gw])
                ct_sb.append(t)

            parts = spool.tile([P, N_BLOCKS], mybir.dt.float32, tag="parts")

            act_scale = 2.0 * SCALE / (FP8_SCALE * FP8_SCALE)
            dve_a = A_EXP * act_scale

            for b in range(B_COLS):
                bw = min(BW, C - b * BW)
                n_sl = (bw + CW - 1) // CW
                g = (b * BW) // GW          # source strip
                goff = (b * BW) % GW        # column offset within the strip
                for m in range(M_TILES):
                    blk_idx = b * M_TILES + m
                    ps = ppool.tile([P, BW], mybir.dt.float32, tag="ps")
                    for k in range(K2):
                        for j in range(n_sl):
                            w = min(CW, bw - j * CW)
                            if g == 0:
                                rhs = c0_sb[goff // CW + j][:, k, :, :w]
                            else:
                                co = goff + j * CW
                                rhs = ct_sb[g][:, k, :, co : co + w]
                            nc.tensor.matmul(
                                ps[:, j * CW : j * CW + w],
                                ft_sb[:, k, :, m * P : (m + 1) * P],
                                rhs,
                                start=(k == 0),
                                stop=(k == K2 - 1),
                                perf_mode=mybir.MatmulPerfMode.DoubleRow,
                                skip_group_check=True,
                            )
                    acc_ap = parts[:, blk_idx : blk_idx + 1]
                    if _is_dve(blk_idx):
                        it = ipool.tile([P, BW], mybir.dt.int32, tag="it")
                        nc.vector.tensor_scalar(
                            it[:, :bw],
                            ps[:, :bw],
                            dve_a,
                            ab_sb[:, 1, m : m + 1],
                            op0=mybir.AluOpType.mult,
                            op1=mybir.AluOpType.add,
                        )
                        itf = it[:, :bw].bitcast(mybir.dt.float32)
                        h = bw // 2
                        nc.vector.scalar_tensor_tensor(
                            itf[:, :h],
                            itf[:, :h],
                            1.0,
                            itf[:, h : 2 * h],
                            op0=mybir.AluOpType.mult,
                            op1=mybir.AluOpType.add,
                            accum_out=acc_ap,
                        )
                    else:
                        et = epool.tile([P, BW], mybir.dt.bfloat16, tag="exp")
                        nc.scalar.activation(
                            et[:, :bw],
                            ps[:, :bw],
                            mybir.ActivationFunctionType.Exp,
                            bias=ab_sb[:, 0, m : m + 1],
                            scale=act_scale,
                            accum_out=acc_ap,
                        )
            nc.sync.dma_start(out=rs_d, in_=parts[:])

    nc.compile()
    return nc


def _get_compiled():
    global _compiled
    if _compiled is None:
        _compiled = _build()
    return _compiled


def _l2n(x):
    n = np.sqrt(np.einsum("nd,nd->n", x, x, dtype=np.float32), dtype=np.float32)
    xh = x / np.maximum(n, np.float32(EPS))[:, None]
    sq = np.einsum("nd,nd->n", xh, xh, dtype=np.float32)
    return xh.astype(np.float32), sq.astype(np.float32)


def _pack_dr(xt):
    """[D, W] fp32 (pre-scaled) -> DoubleRow fp8 [P, K2, 2, W]:
    row d = k*256 + i*128 + p  ->  out[p, k, i]."""
    d, w = xt.shape
    return np.ascontiguousarray(
        xt.reshape(K2, 2, P, w).transpose(2, 0, 1, 3)
    ).astype(FP8)


def _pack_ct(xt):
    """[D, C] fp32 (pre-scaled) -> (strip-0 chunks 4 x [P, K2, 2, CW],
    strips 1.. [G-1, P, K2, 2, GW], last zero-padded)."""
    c0 = [
        np.ascontiguousarray(_pack_dr(xt[:, j * CW : (j + 1) * CW]))
        for j in range(GW // CW)
    ]
    ctr = np.zeros((G_TILES - 1, P, K2, 2, GW), dtype=FP8)
    for g in range(1, G_TILES):
        gw = min(GW, C - g * GW)
        ctr[g - 1, :, :, :, :gw] = _pack_dr(xt[:, g * GW : g * GW + gw])
    return c0, ctr


def _combine_rs(rs):
    """[P, B*M] per-core output (col = blk_idx = b*M + m) -> per-row sums
    [NS] (n = m*128 + p).  Schraudolph columns are pre-divided by R_EXP."""
    scaled = rs.astype(np.float64)
    for idx in range(N_BLOCKS):
        if _is_dve(idx):
            scaled[:, idx] /= R_EXP
    out = scaled.reshape(P, B_COLS, M_TILES).sum(axis=1)
    return out.T.reshape(NS)


def kernel(features, labels, centers, bias):
    features = np.asarray(features, dtype=np.float32)
    centers = np.asarray(centers, dtype=np.float32)
    bias = np.asarray(bias, dtype=np.float32)
    labels_i = np.asarray(labels).astype(np.int64)

    fh, f2 = _l2n(features)          # [N, D], [N]
    ch, c2 = _l2n(centers)           # [C, D], [C]

    c0_8, ct8 = _pack_ct(ch.T * np.float32(FP8_SCALE))
    abias_full = (-SCALE * (f2 + np.float32(1.0))).astype(np.float32)

    in_maps = []
    for i in range(N_CORES):
        sl = slice(i * NS, (i + 1) * NS)
        ft8 = _pack_dr(fh[sl].T * np.float32(FP8_SCALE))    # [P, K2, 2, NS]
        ab = np.ascontiguousarray(
            abias_full[sl].reshape(M_TILES, P).T
        )  # [P, M_TILES], n = m*128 + p
        ab2 = np.stack(
            [ab, (A_EXP * ab.astype(np.float64) + B_EXP).astype(np.float32)],
            axis=1,
        )  # [P, 2, M_TILES]
        im = {"ct": ct8, "ft": ft8, "ab": np.ascontiguousarray(ab2)}
        for j in range(GW // CW):
            im[f"c0{j}"] = c0_8[j]
        in_maps.append(im)

    nc = _get_compiled()
    global LAST_RESULTS
    LAST_RESULTS = run_bass_kernel_spmd(nc, in_maps, core_ids=list(range(N_CORES)))

    rowsum = np.concatenate(
        [_combine_rs(LAST_RESULTS.results[i]["rs"]) for i in range(N_CORES)]
    ).astype(np.float64)

    # residual correction for the |c_k|^2 ~= 1 fold (mean of exp(-5*(c2-1)))
    wmean = np.exp(-SCALE * (c2.astype(np.float64) - 1.0)).mean()
    rowsum *= wmean

    # exact per-row label terms (fp32 inputs, fp64 math)
    cl = ch[labels_i]                                        # [N, D]
    dot = np.einsum("nd,nd->n", fh.astype(np.float64), cl.astype(np.float64))
    dis_l = -SCALE * (f2.astype(np.float64) + c2[labels_i].astype(np.float64) - 2.0 * dot)
    pos = dis_l + bias[labels_i, 0].astype(np.float64)

    num = np.exp(pos)
    den = rowsum - np.exp(dis_l) + num
    logits = np.log(den) - pos
    variance = np.var(pos, ddof=1)
    loss = logits.mean() + variance
    return (np.float32(loss), np.float32(variance))


# revision 14
# speedup vs baseline: 1.2000x; 1.0675x over previous
"""Trainium2 Bass kernel for nn_CenterLossNet (center-loss softmax over classes).

Math (reference):
    f = l2_normalize(features); c = l2_normalize(centers)
    dis[n,k]  = -5 * (|f_n|^2 + |c_k|^2 - 2 f_n.c_k)        # [N, C]
    pos[n]    = dis[n, labels[n]] + bias[labels[n]]
    den[n]    = sum_k exp(dis[n,k]) - exp(dis[n,l_n]) + exp(pos[n])
    loss      = mean(log(den) - pos) + var(pos, ddof=1);  returns (loss, var)

Device does the heavy part: S = f_hat @ c_hat.T (8192x10000x512 matmul in
fp8e4m3 DoubleRow perf mode, operands pre-scaled by 2^9) fused with the
row-sum of exp(10*S + ab_n).  The exp+sum of each [128, 2048] PSUM megatile
goes to one of two engines so neither is the bottleneck (the PE matmul is):

  - ~2/3 of megatiles: scalar-engine ACTIVATE Exp with accum_out (the
    row-sum accumulates for free during the activation; +1 accumulator
    read per tile).
  - ~1/3 of megatiles: vector-engine Schraudolph fast-exp - one
    tensor_scalar writes int32(A*dis + B) (whose bit pattern read as fp32
    approximates exp(dis) to ~2%), a second tensor_scalar row-sums the
    bitcast tile.  The systematic (1+f)/2^f excess is exactly
    E = 1/(2 ln^2 2) under the (empirically uniform) mantissa-fraction
    distribution, so the host divides those partial sums by R; the residual
    per-row fluctuation is ~1e-4 relative.

Everything O(N) or O(C) runs on host in fp64, so pos/variance use exact
fp32 inputs.  Sharding: data-parallel over batch N across 8 cores; centers
replicated.  The per-class |c_k|^2 term is folded as exactly 1.0 with a
host-side mean-residual correction; pos[n] uses the exact per-label norms.
"""

import numpy as np
import ml_dtypes

import concourse.bacc as bacc
import concourse.mybir as mybir
import concourse.tile as tile
from concourse.bass_utils import run_bass_kernel_spmd

N, C, D = 8192, 10000, 512
N_CORES = 8
NS = N // N_CORES       # 1024 rows per core
P = 128                 # partitions
M_TILES = NS // P       # 8 row tiles per core
K2 = D // (2 * P)       # 2 DoubleRow contraction tiles (256 rows each)
CW = 512                # matmul free-dim tile (one PSUM bank of fp32)
GW = 2048               # DRAM strip width (4 x 512-col chunks)
G_TILES = (C + GW - 1) // GW  # 5 (4 x 2048 + 1808)
BW = 1024               # PSUM block width: 2 banks; 4 blocks ping-pong in PSUM
B_COLS = (C + BW - 1) // BW   # 10 column blocks (9 x 1024 + 784)
N_BLOCKS = B_COLS * M_TILES   # 80 blocks per core
SCALE = 5.0
EPS = 1e-12
FP8_SCALE = 512.0       # 2^9: keeps |values| <= ~120 within e4m3 normal range
FP8 = ml_dtypes.float8_e4m3

# Schraudolph fast-exp constants: int32(A*x + B) bitcast to fp32 ~= exp(x).
A_EXP = float(2.0**23 / np.log(2.0))
B_EXP = float(127 * 2**23)
R_EXP = float(1.0 / (2.0 * np.log(2.0) ** 2))   # E[(1+f)/2^f], f ~ U[0,1)

_compiled = None
LAST_RESULTS = None

# Per-block consumer assignment: 36 of 80 blocks go to the DVE Schraudolph
# path (evenly spaced), the rest to the scalar-engine ACTIVATE path.  The
# DVE reduce streams the two halves of the bitcast tile through one
# scalar_tensor_tensor add whose accum_out sums both (half the elements
# streamed).  On alternating DVE blocks a gpsimd tensor_tensor pre-folds
# the halves first so the DVE op streams only a quarter - keeping all of
# PE / ACT / DVE / gpsimd inside the PE-paced window.
N_DVE = 36
_dve_list = sorted({round(i * N_BLOCKS / N_DVE) for i in range(N_DVE)})
DVE_SET = frozenset(_dve_list)
FOLD_SET = frozenset(_dve_list[::2])
assert len(DVE_SET) == N_DVE


def _is_dve(idx: int) -> bool:
    return idx in DVE_SET


def _build():
    nc = bacc.Bacc(
        "TRN2",
        target_bir_lowering=False,
        debug=False,
        enable_asserts=False,
        num_devices=N_CORES,
    )
    # strip 0 is stored as four 512-column chunks (2 KB per partition each)
    # so the first matmuls can start as soon as chunk 0 lands
    c0_d = [
        nc.dram_tensor(f"c0{j}", [P, K2, 2, CW], mybir.dt.float8e4, kind="ExternalInput").ap()
        for j in range(GW // CW)
    ]
    ct_d = nc.dram_tensor(
        "ct", [G_TILES - 1, P, K2, 2, GW], mybir.dt.float8e4, kind="ExternalInput"
    ).ap()
    ft_d = nc.dram_tensor(
        "ft", [P, K2, 2, NS], mybir.dt.float8e4, kind="ExternalInput"
    ).ap()
    # ab[:, 0, m] = ACT exp bias; ab[:, 1, m] = A*ab + B (Schraudolph bias)
    ab_d = nc.dram_tensor("ab", [P, 2, M_TILES], mybir.dt.float32, kind="ExternalInput").ap()
    # per-(b,m) partial row-sums (col = b*M + m); host does the final combine
    rs_d = nc.dram_tensor(
        "rs", [P, N_BLOCKS], mybir.dt.float32, kind="ExternalOutput"
    ).ap()

    with tile.TileContext(nc) as tc:
        with (
            tc.tile_pool(name="cpool", bufs=1) as cpool,
            tc.tile_pool(name="spool", bufs=1) as spool,
            tc.tile_pool(name="epool", bufs=1) as epool,
            tc.tile_pool(name="ipool", bufs=3) as ipool,
            tc.tile_pool(name="ppool", bufs=4, space="PSUM") as ppool,
        ):
            # warm the PE clock (HAM) with throwaway DoubleRow matmuls on a
            # zeroed tile while the first input DMAs are in flight
            z8 = spool.tile([P, 2, CW], mybir.dt.float8e4, tag="z8")
            nc.gpsimd.memset(z8[:], 0.0)
            wps = ppool.tile([P, CW], mybir.dt.float32, tag="ps", name="wps")
            for _ in range(12):
                nc.tensor.matmul(
                    wps[:],
                    z8[:, :, 0:P],
                    z8[:],
                    start=True,
                    stop=True,
                    perf_mode=mybir.MatmulPerfMode.DoubleRow,
                    skip_group_check=True,
                )

            # critical prefix on the sync ring, in first-use order
            ft_sb = cpool.tile([P, K2, 2, NS], mybir.dt.float8e4, tag="ft")
            nc.sync.dma_start(out=ft_sb[:], in_=ft_d)

            c0_sb = []
            for j in range(GW // CW):
                t = cpool.tile([P, K2, 2, CW], mybir.dt.float8e4, tag=f"c0{j}")
                c0_sb.append(t)
            nc.sync.dma_start(out=c0_sb[0][:], in_=c0_d[0])
            nc.sync.dma_start(out=c0_sb[1][:], in_=c0_d[1])

            ab_sb = spool.tile([P, 2, M_TILES], mybir.dt.float32, tag="ab")
            nc.sync.dma_start(out=ab_sb[:], in_=ab_d)

            nc.sync.dma_start(out=c0_sb[2][:], in_=c0_d[2])
            nc.sync.dma_start(out=c0_sb[3][:], in_=c0_d[3])

            # remaining strips: one DMA per strip, FIFO behind the prefix
            ct_sb = [None]
            for g in range(1, G_TILES):
                gw = min(GW, C - g * GW)
                t = cpool.tile(
                    [P, K2, 2, GW], mybir.dt.float8e4, tag=f"ct{g}", name=f"ct{g}"
                )
                nc.sync.dma_start(out=t[:, :, :, :gw], in_=ct_d[g - 1][:, :, :, :gw])
                ct_sb.append(t)

            parts = spool.tile([P, N_BLOCKS], mybir.dt.float32, tag="parts")

            act_scale = 2.0 * SCALE / (FP8_SCALE * FP8_SCALE)
            dve_a = A_EXP * act_scale

            for b in range(B_COLS):
                bw = min(BW, C - b * BW)
                n_sl = (bw + CW - 1) // CW
                g = (b * BW) // GW          # source strip
                goff = (b * BW) % GW        # column offset within the strip
                for m in range(M_TILES):
                    blk_idx = b * M_TILES + m
                    ps = ppool.tile([P, BW], mybir.dt.float32, tag="ps")
                    for k in range(K2):
                        for j in range(n_sl):
                            w = min(CW, bw - j * CW)
                            if g == 0:
                                rhs = c0_sb[goff // CW + j][:, k, :, :w]
                            else:
                                co = goff + j * CW
                                rhs = ct_sb[g][:, k, :, co : co + w]
                            nc.tensor.matmul(
                                ps[:, j * CW : j * CW + w],
                                ft_sb[:, k, :, m * P : (m + 1) * P],
                                rhs,
                                start=(k == 0),
                                stop=(k == K2 - 1),
                                perf_mode=mybir.MatmulPerfMode.DoubleRow,
                                skip_group_check=True,
                            )
                    acc_ap = parts[:, blk_idx : blk_idx + 1]
                    if _is_dve(blk_idx):
                        it = ipool.tile([P, BW], mybir.dt.int32, tag="it")
                        nc.vector.tensor_scalar(
                            it[:, :bw],
                            ps[:, :bw],
                            dve_a,
                            ab_sb[:, 1, m : m + 1],
                            op0=mybir.AluOpType.mult,
                            op1=mybir.AluOpType.add,
                        )
                        itf = it[:, :bw].bitcast(mybir.dt.float32)
                        h = bw // 2
                        if blk_idx in FOLD_SET:
                            nc.gpsimd.tensor_tensor(
                                itf[:, :h],
                                itf[:, :h],
                                itf[:, h : 2 * h],
                                op=mybir.AluOpType.add,
                            )
                            h = h // 2
                        nc.vector.scalar_tensor_tensor(
                            itf[:, :h],
                            itf[:, :h],
                            1.0,
                            itf[:, h : 2 * h],
                            op0=mybir.AluOpType.mult,
                            op1=mybir.AluOpType.add,
                            accum_out=acc_ap,
                        )
                    else:
                        et = epool.tile([P, BW], mybir.dt.bfloat16, tag="exp")
                        nc.scalar.activation(
                            et[:, :bw],
                            ps[:, :bw],
                            mybir.ActivationFunctionType.Exp,
                            bias=ab_sb[:, 0, m : m + 1],
                            scale=act_scale,
                            accum_out=acc_ap,
                        )
            nc.sync.dma_start(out=rs_d, in_=parts[:])

    nc.compile()
    return nc


def _get_compiled():
    global _compiled
    if _compiled is None:
        _compiled = _build()
    return _compiled


def _l2n(x):
    n = np.sqrt(np.einsum("nd,nd->n", x, x, dtype=np.float32), dtype=np.float32)
    xh = x / np.maximum(n, np.float32(EPS))[:, None]
    sq = np.einsum("nd,nd->n", xh, xh, dtype=np.float32)
    return xh.astype(np.float32), sq.astype(np.float32)


def _pack_dr(xt):
    """[D, W] fp32 (pre-scaled) -> DoubleRow fp8 [P, K2, 2, W]:
    row d = k*256 + i*128 + p  ->  out[p, k, i]."""
    d, w = xt.shape
    return np.ascontiguousarray(
        xt.reshape(K2, 2, P, w).transpose(2, 0, 1, 3)
    ).astype(FP8)


def _pack_ct(xt):
    """[D, C] fp32 (pre-scaled) -> (strip-0 chunks 4 x [P, K2, 2, CW],
    strips 1.. [G-1, P, K2, 2, GW], last zero-padded)."""
    c0 = [
        np.ascontiguousarray(_pack_dr(xt[:, j * CW : (j + 1) * CW]))
        for j in range(GW // CW)
    ]
    ctr = np.zeros((G_TILES - 1, P, K2, 2, GW), dtype=FP8)
    for g in range(1, G_TILES):
        gw = min(GW, C - g * GW)
        ctr[g - 1, :, :, :, :gw] = _pack_dr(xt[:, g * GW : g * GW + gw])
    return c0, ctr


def _combine_rs(rs):
    """[P, B*M] per-core output (col = blk_idx = b*M + m) -> per-row sums
    [NS] (n = m*128 + p).  Schraudolph columns are pre-divided by R_EXP."""
    scaled = rs.astype(np.float64)
    for idx in range(N_BLOCKS):
        if _is_dve(idx):
            scaled[:, idx] /= R_EXP
    out = scaled.reshape(P, B_COLS, M_TILES).sum(axis=1)
    return out.T.reshape(NS)


def kernel(features, labels, centers, bias):
    features = np.asarray(features, dtype=np.float32)
    centers = np.asarray(centers, dtype=np.float32)
    bias = np.asarray(bias, dtype=np.float32)
    labels_i = np.asarray(labels).astype(np.int64)

    fh, f2 = _l2n(features)          # [N, D], [N]
    ch, c2 = _l2n(centers)           # [C, D], [C]

    c0_8, ct8 = _pack_ct(ch.T * np.float32(FP8_SCALE))
    abias_full = (-SCALE * (f2 + np.float32(1.0))).astype(np.float32)

    in_maps = []
    for i in range(N_CORES):
        sl = slice(i * NS, (i + 1) * NS)
        ft8 = _pack_dr(fh[sl].T * np.float32(FP8_SCALE))    # [P, K2, 2, NS]
        ab = np.ascontiguousarray(
            abias_full[sl].reshape(M_TILES, P).T
        )  # [P, M_TILES], n = m*128 + p
        ab2 = np.stack(
            [ab, (A_EXP * ab.astype(np.float64) + B_EXP).astype(np.float32)],
            axis=1,
        )  # [P, 2, M_TILES]
        im = {"ct": ct8, "ft": ft8, "ab": np.ascontiguousarray(ab2)}
        for j in range(GW // CW):
            im[f"c0{j}"] = c0_8[j]
        in_maps.append(im)

    nc = _get_compiled()
    global LAST_RESULTS
    LAST_RESULTS = run_bass_kernel_spmd(nc, in_maps, core_ids=list(range(N_CORES)))

    rowsum = np.concatenate(
        [_combine_rs(LAST_RESULTS.results[i]["rs"]) for i in range(N_CORES)]
    ).astype(np.float64)

    # residual correction for the |c_k|^2 ~= 1 fold (mean of exp(-5*(c2-1)))
    wmean = np.exp(-SCALE * (c2.astype(np.float64) - 1.0)).mean()
    rowsum *= wmean

    # exact per-row label terms (fp32 inputs, fp64 math)
    cl = ch[labels_i]                                        # [N, D]
    dot = np.einsum("nd,nd->n", fh.astype(np.float64), cl.astype(np.float64))
    dis_l = -SCALE * (f2.astype(np.float64) + c2[labels_i].astype(np.float64) - 2.0 * dot)
    pos = dis_l + bias[labels_i, 0].astype(np.float64)

    num = np.exp(pos)
    den = rowsum - np.exp(dis_l) + num
    logits = np.log(den) - pos
    variance = np.var(pos, ddof=1)
    loss = logits.mean() + variance
    return (np.float32(loss), np.float32(variance))
